# revision 19
# baseline (speedup 1.0000x reference)
"""Trainium2 Bass kernel for nn_Decoder_74835510165950 (sparse_attention).

Single-query attention decoder over B=64, N=2000, H=128, 8 heads.
Data-parallel over 8 NeuronCores: 8 batches per core.

v4 design:
  - Only X^T fp16 is DMA'd (4.2MB/core, N padded to 2048, chunk-major
    -> 8KB contiguous runs, ~350GB/s). Small tensors go FIRST on the
    same queue so the Q-path isn't starved behind the bulk load.
  - X-natural (for attnV) is rebuilt on device: 8 PE transposes into one
    fp16 [128,1024] PSUM bank per 128-col n-tile, then a single copy,
    alternating Scalar/Vector engines so neither serializes the PE.
  - All 8 batches packed per PSUM tile (scores rows 8b..8b+8, pointer
    row b); -60000 masking via one extra matmul per chunk with a host
    one-hot rhs; a 9th row masks the 48 pad columns.
  - fp16 streams everywhere (PSUM fp32); softmax exp biased by -8 so E
    fits fp16; 1/sum applied to u after attnV; logit_Wk^T@Wc and
    logit_Wk^T@bc folded on the host so the output head is one matmul.
  - Fully chunk-pipelined: DMA -> scores -> exp -> xnat rebuild + E^T ->
    attnV per 512-col chunk; only the small chain + pointer phase trail
    the last chunk.
"""
import sys

if "/opt/trn_rl_repo" not in sys.path:
    sys.path.insert(0, "/opt/trn_rl_repo")

import math
import numpy as np

import concourse.bass as bass
import concourse.tile as tile
from concourse import bacc, mybir
from concourse.bass_utils import run_bass_kernel_spmd

F32 = mybir.dt.float32
F16 = mybir.dt.float16

N_CORES = 8
B_CORE = 8          # batches per core
N = 2000
N2 = 2048           # padded
H = 128
NH = 8              # heads
HD = 16             # head dim
NCH = 4             # chunks
CW = 512            # chunk width (= one psum bank of fp32)
NJ = 16             # 128-col n-tiles (N2 / 128)
EXP_BIAS = -8.0     # uniform shift inside softmax exp (cancels in ratio)
MNEG = -60000.0

# wpack column layout (all fp16, [128, x])
_WCOLS = {"wqgT": 0, "wsumT": 128, "wk": 256, "wvT": 384, "wlc": 512,
          "id": 640, "hm": 768, "leT": 776, "clsT": 784}
WPACK_W = 792
SM_W = N2 + 2 * H   # small9 row width

_CACHE = {}


def build():
    nc = bacc.Bacc("TRN2", target_bir_lowering=False, debug=False)

    xtp = nc.dram_tensor("xtp", [H, NCH, B_CORE, CW], F16, kind="ExternalInput")
    wpack = nc.dram_tensor("wpack", [H, WPACK_W], F16, kind="ExternalInput")
    small9 = nc.dram_tensor("small9", [B_CORE + 1, SM_W], F16,
                            kind="ExternalInput")
    blcd = nc.dram_tensor("blcd", [H, 1], F32, kind="ExternalInput")

    probs = nc.dram_tensor("probs", [B_CORE, N], F32, kind="ExternalOutput")

    AF = mybir.ActivationFunctionType

    with tile.TileContext(nc) as tc:
        with (
            tc.tile_pool(name="wts", bufs=1) as wts,
            tc.tile_pool(name="xt", bufs=1) as xtp_p,
            tc.tile_pool(name="xn", bufs=1) as xnp_p,
            tc.tile_pool(name="big", bufs=1) as bigp,
            tc.tile_pool(name="sm", bufs=1) as smp,
            tc.tile_pool(name="ps_sc", bufs=2, space="PSUM") as psc,
            tc.tile_pool(name="ps_tr", bufs=3, space="PSUM") as pst,
            tc.tile_pool(name="ps_u", bufs=1, space="PSUM") as psu,
            tc.tile_pool(name="ps_sm", bufs=1, space="PSUM") as pss,
        ):
            # ---------- DMAs: smalls first, then X^T chunks ----------
            wpack_s = wts.tile([H, WPACK_W], F16, tag="wpack")
            nc.sync.dma_start(wpack_s[:], wpack[:])
            small9_s = wts.tile([B_CORE + 1, SM_W], F16, tag="small9")
            nc.sync.dma_start(small9_s[:], small9[:])
            blc_s = wts.tile([H, 1], F32, tag="blc")
            nc.sync.dma_start(blc_s[:], blcd[:])

            def wcol(name, w):
                c0 = _WCOLS[name]
                return wpack_s[:, c0:c0 + w]

            wqgT16, wsumT16 = wcol("wqgT", H), wcol("wsumT", H)
            wk16 = wcol("wk", H)
            wvT16 = wcol("wvT", H)
            wlc16 = wcol("wlc", H)
            id16 = wcol("id", H)
            hm16 = wcol("hm", NH)
            leT16 = wcol("leT", B_CORE)
            clsT16 = wcol("clsT", B_CORE)
            oh9 = small9_s[:, 0:N2]
            mnegA9 = small9_s[:, N2:N2 + H]
            mnegP9 = small9_s[:, N2 + H:N2 + 2 * H]

            xT = xtp_p.tile([H, NCH, B_CORE, CW], F16, tag="xT")
            for c in range(NCH):
                nc.sync.dma_start(xT[:, c, :, :], xtp[:, c, :, :])

            def sps():
                return pss.tile([H, H], F32, tag="smps", name="smps")

            def tps():
                return pst.tile([H, 8 * H], F16, tag="tps", name="tps")

            # ---------- Q path (runs during X DMA) ----------
            # flat stationaries: window b = cols 129b..129b+128. rp8f holds
            # R cols at 137b+h (=> window-local 8b+h); wp8f holds w2 col at
            # 130b (=> window-local b). One strided copy builds each.
            rp8f = smp.tile([H, 137 * B_CORE], F16, tag="rp8f")
            nc.gpsimd.memset(rp8f[:], 0.0)
            wp8f = smp.tile([H, 130 * B_CORE], F16, tag="wp8f")
            nc.gpsimd.memset(wp8f[:], 0.0)

            def rp8(b):
                return rp8f[:, 129 * b:129 * b + H]

            def wp8(b):
                return wp8f[:, 129 * b:129 * b + H]
            eb = smp.tile([H, 1], F32, tag="eb")
            nc.vector.memset(eb[:], EXP_BIAS)
            z16 = smp.tile([H, 1], F16, tag="z16")
            nc.vector.memset(z16[:], 0.0)

            q_ps = sps()[:, 0:B_CORE]
            nc.tensor.matmul(q_ps, wqgT16, clsT16, start=True, stop=False)
            nc.tensor.matmul(q_ps, wsumT16, leT16, start=False, stop=True)
            q_s = smp.tile([H, B_CORE], F32, tag="q_s")
            nc.vector.tensor_copy(q_s[:], q_ps)

            qtil = smp.tile([H, B_CORE * NH], F16, tag="qtil")
            for b in range(B_CORE):
                if b % 2 == 0:
                    nc.vector.tensor_scalar_mul(
                        qtil[:, NH * b:NH * (b + 1)], hm16, q_s[:, b:b + 1])
                else:
                    nc.scalar.activation(
                        qtil[:, NH * b:NH * (b + 1)], hm16, AF.Identity,
                        bias=z16[:, 0:1], scale=q_s[:, b:b + 1])
            r_ps = sps()[:, 0:B_CORE * NH]
            nc.tensor.matmul(r_ps, wk16, qtil[:], start=True, stop=True)
            nc.vector.tensor_copy(
                rp8f[:].rearrange("p (b k) -> p b k", k=137)[:, :, 0:NH],
                r_ps[:].rearrange("p (b h) -> p b h", h=NH))

            # HAM warm-up: dummy matmuls fill the PE while the first X^T
            # chunk is still in flight, so chunk 0 runs at 2.4GHz.
            dmy = psc.tile([H, CW], F32, tag="scps", name="dmy")
            for _ in range(8):
                nc.tensor.matmul(dmy[:], id16, wpack_s[:, 0:CW],
                                 start=True, stop=True)

            # ---------- main chunk pipeline ----------
            E = bigp.tile([H, N2], F16, tag="E")
            sums = smp.tile([H, NCH], F32, tag="sums")
            et = bigp.tile([H, NJ, H], F16, tag="et")
            xn = xnp_p.tile([H, NJ, B_CORE, H], F16, tag="xn")
            u_ps = [psu.tile([H, 4 * H], F32, tag=f"ups{g}", name=f"ups{g}")
                    for g in range(2)]

            for c in range(NCH):
                xTc = xT[:, c, :, :]
                cs = slice(CW * c, CW * (c + 1))
                # scores
                sc = psc.tile([H, CW], F32, tag="scps", name="sc")
                for b in range(B_CORE):
                    nc.tensor.matmul(sc[:], rp8(b), xTc[:, b, :],
                                     start=(b == 0), stop=False)
                nc.tensor.matmul(sc[:], mnegA9, oh9[:, cs],
                                 start=False, stop=True)
                nc.scalar.activation(
                    E[:, cs], sc[:], AF.Exp,
                    bias=eb[:, 0:1], scale=1.0, accum_out=sums[:, c:c + 1])

                # rebuild X-natural for this chunk (PE transposes, f16 PSUM)
                for i in range(4):
                    j = 4 * c + i
                    xb = tps()
                    for b in range(B_CORE):
                        nc.tensor.transpose(
                            xb[:, H * b:H * (b + 1)],
                            xTc[:, b, H * i:H * (i + 1)], id16)
                    dst = xn[:, j, :, :].rearrange("p a c -> p (a c)")
                    if i % 2 == 0:
                        nc.vector.tensor_copy(dst, xb[:])
                    else:
                        nc.scalar.activation(dst, xb[:], AF.Identity,
                                             bias=z16[:, 0:1], scale=1.0)

                # E^T for this chunk
                ep = tps()[:, 0:4 * H]
                for i in range(4):
                    j = 4 * c + i
                    nc.tensor.transpose(
                        ep[:, H * i:H * (i + 1)], E[:, H * j:H * (j + 1)],
                        id16)
                nc.vector.tensor_copy(
                    et[:, 4 * c:4 * c + 4, :]
                    .rearrange("p a c -> p (a c)"), ep)

                # attnV partial accumulation for this chunk
                for i in range(4):
                    j = 4 * c + i
                    for g in range(2):
                        nc.tensor.matmul(
                            u_ps[g][:], et[:, j, :],
                            xn[:, j, 4 * g:4 * g + 4, :]
                            .rearrange("p a c -> p (a c)"),
                            start=(j == 0), stop=(j == NJ - 1))

            s01 = smp.tile([H, 2], F32, tag="s01")
            nc.vector.tensor_add(s01[:, 0:1], sums[:, 0:1], sums[:, 1:2])
            nc.vector.tensor_add(s01[:, 1:2], sums[:, 2:3], sums[:, 3:4])
            stot = smp.tile([H, 1], F32, tag="stot")
            nc.vector.tensor_add(stot[:], s01[:, 0:1], s01[:, 1:2])
            rcp = smp.tile([H, 1], F32, tag="rcp")
            nc.vector.reciprocal(rcp[:], stot[:])

            us = smp.tile([H, 2, 4 * H], F16, tag="us")
            for g in range(2):
                nc.vector.tensor_scalar_mul(us[:, g, :], u_ps[g][:],
                                            rcp[:, 0:1])

            # ---------- uT, v, w2 ----------
            uT = smp.tile([H, B_CORE * NH], F16, tag="uT")
            for b in range(B_CORE):
                o = H * (b % 4)
                ps = tps()[:, 0:H]
                nc.tensor.transpose(ps, us[:, b // 4, o:o + H], id16)
                nc.vector.tensor_copy(
                    uT[:, NH * b:NH * (b + 1)], ps[:, NH * b:NH * (b + 1)])

            v_ps = sps()[0:B_CORE, :]
            for h in range(NH):
                nc.tensor.matmul(
                    v_ps[:, HD * h:HD * (h + 1)],
                    uT[:].rearrange("p (b h) -> p h b", h=NH)[:, h, :],
                    wvT16[:, HD * h:HD * (h + 1)],
                    start=True, stop=True)
            v_s = smp.tile([B_CORE, H], F16, tag="v_s")
            nc.vector.tensor_copy(v_s[:], v_ps)

            vt_ps = tps()[:, 0:B_CORE]
            nc.tensor.transpose(vt_ps, v_s[:], id16[0:B_CORE, 0:B_CORE])
            vT = smp.tile([H, B_CORE], F16, tag="vT")
            nc.vector.tensor_copy(vT[:], vt_ps)

            # w2 = (ls * logit_Wk^T Wc) v + ls * logit_Wk^T bc  (host-folded)
            w2_ps = sps()[:, 0:B_CORE]
            nc.tensor.matmul(w2_ps, wlc16, vT[:], start=True, stop=True)
            w2_s = smp.tile([H, B_CORE], F16, tag="w2_s")
            nc.scalar.activation(
                w2_s[:], w2_ps, AF.Identity, bias=blc_s[:, 0:1], scale=1.0)
            nc.vector.tensor_copy(
                wp8f[:].rearrange("p (b k) -> p b k", k=130)[:, :, 0:1],
                w2_s[:].rearrange("p (b o) -> p b o", o=1))

            # ---------- pointer scores -> tanh -> exp -> norm ----------
            tp = bigp.tile([B_CORE, N2], F16, tag="tp")
            e2 = bigp.tile([B_CORE, N2], F32, tag="e2")
            s2s = smp.tile([B_CORE, NCH], F32, tag="s2s")
            s2p = smp.tile([B_CORE, 2], F32, tag="s2p")
            for c in range(NCH):
                xTc = xT[:, c, :, :]
                cs = slice(CW * c, CW * (c + 1))
                pc = psc.tile([H, CW], F32, tag="scps", name="pc")
                for b in range(B_CORE):
                    nc.tensor.matmul(pc[:], wp8(b), xTc[:, b, :],
                                     start=(b == 0), stop=False)
                nc.tensor.matmul(pc[:], mnegP9, oh9[:, cs],
                                 start=False, stop=True)
                nc.scalar.activation(tp[:, cs], pc[0:B_CORE, :], AF.Tanh)
                nc.scalar.activation(
                    e2[:, cs], tp[:, cs], AF.Exp,
                    bias=0.0, scale=10.0, accum_out=s2s[:, c:c + 1])
                if c == 1:
                    nc.vector.tensor_add(s2p[:, 0:1], s2s[:, 0:1],
                                         s2s[:, 1:2])
                if c == 3:
                    nc.vector.tensor_add(s2p[:, 1:2], s2s[:, 2:3],
                                         s2s[:, 3:4])

            s2tot = smp.tile([B_CORE, 1], F32, tag="s2tot")
            nc.vector.tensor_add(s2tot[:], s2p[:, 0:1], s2p[:, 1:2])
            rcp2 = smp.tile([B_CORE, 1], F32, tag="rcp2")
            nc.vector.reciprocal(rcp2[:], s2tot[:])
            NHALF = 1056
            nc.scalar.activation(e2[:, NHALF:N], e2[:, NHALF:N], AF.Identity,
                                 bias=0.0, scale=rcp2[:, 0:1])
            nc.scalar.dma_start(probs[:, NHALF:N], e2[:, NHALF:N])
            nc.vector.tensor_scalar_mul(e2[:, 0:NHALF], e2[:, 0:NHALF],
                                        rcp2[:, 0:1])
            nc.sync.dma_start(probs[:, 0:NHALF], e2[:, 0:NHALF])

    nc.compile()
    return nc


def _prep_inputs(patch_embeddings, fixed_content_cls, Wq_graph, Wq_first,
                 Wq_last, Wk, Wv, logit_Wk, Wc, bc, last_patch):
    qs = 1.0 / math.sqrt(HD)
    ls = 1.0 / math.sqrt(H)
    f16 = lambda a: np.ascontiguousarray(a, dtype=np.float16)
    f32 = lambda a: np.ascontiguousarray(a, dtype=np.float32)

    hm = np.zeros((H, NH), np.float32)
    for h in range(NH):
        hm[HD * h:HD * (h + 1), h] = qs

    pe = np.asarray(patch_embeddings, dtype=np.float32)
    cls = np.asarray(fixed_content_cls, dtype=np.float32)
    lp = np.asarray(last_patch).astype(np.int64)
    lWk = np.asarray(logit_Wk, dtype=np.float64)
    Wc64 = np.asarray(Wc, dtype=np.float64)

    wpack_base = np.zeros((H, WPACK_W), np.float16)
    wpack_base[:, 0:128] = f16(np.asarray(Wq_graph).T)
    wpack_base[:, 128:256] = f16(np.asarray(Wq_first) + np.asarray(Wq_last)).T
    wpack_base[:, 256:384] = f16(Wk)
    wpack_base[:, 384:512] = f16(np.asarray(Wv).T)
    # w2 = wlc^T v + blc : lhsT = (ls * lWk^T Wc)^T = ls * Wc^T lWk
    wpack_base[:, 512:640] = f16(Wc64.T @ lWk * ls)
    wpack_base[:, 640:768] = np.eye(H, dtype=np.float16)
    wpack_base[:, 768:776] = f16(hm)

    blc = f32((lWk.T @ np.asarray(bc, dtype=np.float64) * ls)[:, None])

    in_maps = []
    for c in range(N_CORES):
        bs = slice(B_CORE * c, B_CORE * (c + 1))
        pec = pe[bs]                        # (8, 2000, 128)
        lp_c = lp[bs]
        wpack = wpack_base.copy()
        wpack[:, 776:784] = f16(pec[np.arange(B_CORE), lp_c].T)   # leT
        wpack[:, 784:792] = f16(cls[bs, 0, :].T)                   # clsT

        pad = np.zeros((B_CORE, N2, H), np.float16)
        pad[:, :N, :] = pec
        # (b, chunk, n, c) -> (c, chunk, b, n)
        xtp = np.ascontiguousarray(
            pad.reshape(B_CORE, NCH, CW, H).transpose(3, 1, 0, 2))

        small9 = np.zeros((B_CORE + 1, SM_W), np.float16)
        small9[np.arange(B_CORE), lp_c] = 1.0                      # one-hot
        small9[B_CORE, N:N2] = 1.0                                 # pad cols
        for b in range(B_CORE):
            small9[b, N2 + NH * b:N2 + NH * (b + 1)] = MNEG        # mnegA9
            small9[b, N2 + H + b] = MNEG                           # mnegP9
        small9[B_CORE, N2:N2 + 2 * H] = MNEG                       # pad row

        m = {"xtp": xtp, "wpack": wpack, "small9": small9, "blcd": blc}
        in_maps.append(m)
    return in_maps


def kernel(trace=False, **inputs):
    if "nc" not in _CACHE:
        _CACHE["nc"] = build()
    nc = _CACHE["nc"]
    in_maps = _prep_inputs(**inputs)
    res = run_bass_kernel_spmd(nc, in_maps, list(range(N_CORES)), trace=trace)
    out = np.concatenate(
        [res.results[c]["probs"].astype(np.float32) for c in range(N_CORES)],
        axis=0)
    if trace:
        return out, res
    return out


# revision 21
# speedup vs baseline: 1.0116x; 1.0116x over previous
"""Trainium2 Bass kernel for nn_Decoder_74835510165950 (sparse_attention).

Single-query attention decoder over B=64, N=2000, H=128, 8 heads.
Data-parallel over 8 NeuronCores: 8 batches per core.

v4 design:
  - Only X^T fp16 is DMA'd (4.2MB/core, N padded to 2048, chunk-major
    -> 8KB contiguous runs, ~350GB/s). Small tensors go FIRST on the
    same queue so the Q-path isn't starved behind the bulk load.
  - X-natural (for attnV) is rebuilt on device: 8 PE transposes into one
    fp16 [128,1024] PSUM bank per 128-col n-tile, then a single copy,
    alternating Scalar/Vector engines so neither serializes the PE.
  - All 8 batches packed per PSUM tile (scores rows 8b..8b+8, pointer
    row b); -60000 masking via one extra matmul per chunk with a host
    one-hot rhs; a 9th row masks the 48 pad columns.
  - fp16 streams everywhere (PSUM fp32); softmax exp biased by -8 so E
    fits fp16; 1/sum applied to u after attnV; logit_Wk^T@Wc and
    logit_Wk^T@bc folded on the host so the output head is one matmul.
  - Fully chunk-pipelined: DMA -> scores -> exp -> xnat rebuild + E^T ->
    attnV per 512-col chunk; only the small chain + pointer phase trail
    the last chunk.
"""
import sys

if "/opt/trn_rl_repo" not in sys.path:
    sys.path.insert(0, "/opt/trn_rl_repo")

import math
import numpy as np

import concourse.bass as bass
import concourse.tile as tile
from concourse import bacc, mybir
from concourse.bass_utils import run_bass_kernel_spmd

F32 = mybir.dt.float32
F16 = mybir.dt.float16

N_CORES = 8
B_CORE = 8          # batches per core
N = 2000
N2 = 2048           # padded
H = 128
NH = 8              # heads
HD = 16             # head dim
NCH = 4             # chunks
CW = 512            # chunk width (= one psum bank of fp32)
NJ = 16             # 128-col n-tiles (N2 / 128)
EXP_BIAS = -8.0     # uniform shift inside softmax exp (cancels in ratio)
MNEG = -60000.0

# wpack column layout (all fp16, [128, x])
_WCOLS = {"wqgT": 0, "wsumT": 128, "wk": 256, "id": 384, "hm": 512,
          "leT": 520, "clsT": 528, "wlv": 536}
WPACK_W = 536 + 8 * 128
SM_W = N2 + 2 * H   # small9 row width

_CACHE = {}


def build():
    nc = bacc.Bacc("TRN2", target_bir_lowering=False, debug=False)

    xtp = nc.dram_tensor("xtp", [H, NCH, B_CORE, CW], F16, kind="ExternalInput")
    wpack = nc.dram_tensor("wpack", [H, WPACK_W], F16, kind="ExternalInput")
    small9 = nc.dram_tensor("small9", [B_CORE + 1, SM_W], F16,
                            kind="ExternalInput")
    blcd = nc.dram_tensor("blcd", [H, 1], F32, kind="ExternalInput")

    probs = nc.dram_tensor("probs", [B_CORE, N], F32, kind="ExternalOutput")

    AF = mybir.ActivationFunctionType

    with tile.TileContext(nc) as tc:
        with (
            tc.tile_pool(name="wts", bufs=1) as wts,
            tc.tile_pool(name="xt", bufs=1) as xtp_p,
            tc.tile_pool(name="xn", bufs=1) as xnp_p,
            tc.tile_pool(name="big", bufs=1) as bigp,
            tc.tile_pool(name="sm", bufs=1) as smp,
            tc.tile_pool(name="ps_sc", bufs=2, space="PSUM") as psc,
            tc.tile_pool(name="ps_tr", bufs=3, space="PSUM") as pst,
            tc.tile_pool(name="ps_u", bufs=1, space="PSUM") as psu,
            tc.tile_pool(name="ps_sm", bufs=1, space="PSUM") as pss,
        ):
            # ---------- DMAs: smalls first, then X^T chunks ----------
            wpack_s = wts.tile([H, WPACK_W], F16, tag="wpack")
            nc.sync.dma_start(wpack_s[:], wpack[:])
            small9_s = wts.tile([B_CORE + 1, SM_W], F16, tag="small9")
            nc.sync.dma_start(small9_s[:], small9[:])
            blc_s = wts.tile([H, 1], F32, tag="blc")
            nc.sync.dma_start(blc_s[:], blcd[:])

            def wcol(name, w):
                c0 = _WCOLS[name]
                return wpack_s[:, c0:c0 + w]

            wqgT16, wsumT16 = wcol("wqgT", H), wcol("wsumT", H)
            wk16 = wcol("wk", H)
            wlv16 = wcol("wlv", NH * H)
            id16 = wcol("id", H)
            hm16 = wcol("hm", NH)
            leT16 = wcol("leT", B_CORE)
            clsT16 = wcol("clsT", B_CORE)
            oh9 = small9_s[:, 0:N2]
            mnegA9 = small9_s[:, N2:N2 + H]
            mnegP9 = small9_s[:, N2 + H:N2 + 2 * H]

            xT = xtp_p.tile([H, NCH, B_CORE, CW], F16, tag="xT")
            HW2 = CW // 2
            nc.sync.dma_start(xT[:, 0, :, 0:HW2], xtp[:, 0, :, 0:HW2])
            nc.sync.dma_start(xT[:, 0, :, HW2:CW], xtp[:, 0, :, HW2:CW])
            for c in range(1, NCH):
                nc.sync.dma_start(xT[:, c, :, :], xtp[:, c, :, :])

            def sps():
                return pss.tile([H, H], F32, tag="smps", name="smps")

            def tps():
                return pst.tile([H, 8 * H], F16, tag="tps", name="tps")

            # ---------- Q path (runs during X DMA) ----------
            # flat stationaries: window b = cols 129b..129b+128. rp8f holds
            # R cols at 137b+h (=> window-local 8b+h); wp8f holds w2 col at
            # 130b (=> window-local b). One strided copy builds each.
            rp8f = smp.tile([H, 137 * B_CORE], F16, tag="rp8f")
            nc.gpsimd.memset(rp8f[:], 0.0)
            wp8f = smp.tile([H, 130 * B_CORE], F16, tag="wp8f")
            nc.gpsimd.memset(wp8f[:], 0.0)

            def rp8(b):
                return rp8f[:, 129 * b:129 * b + H]

            def wp8(b):
                return wp8f[:, 129 * b:129 * b + H]
            eb = smp.tile([H, 1], F32, tag="eb")
            nc.vector.memset(eb[:], EXP_BIAS)
            z16 = smp.tile([H, 1], F16, tag="z16")
            nc.vector.memset(z16[:], 0.0)

            q_ps = sps()[:, 0:B_CORE]
            nc.tensor.matmul(q_ps, wqgT16, clsT16, start=True, stop=False)
            nc.tensor.matmul(q_ps, wsumT16, leT16, start=False, stop=True)
            q_s = smp.tile([H, B_CORE], F32, tag="q_s")
            nc.vector.tensor_copy(q_s[:], q_ps)

            qtil = smp.tile([H, B_CORE * NH], F16, tag="qtil")
            for b in range(B_CORE):
                if b % 2 == 0:
                    nc.vector.tensor_scalar_mul(
                        qtil[:, NH * b:NH * (b + 1)], hm16, q_s[:, b:b + 1])
                else:
                    nc.scalar.activation(
                        qtil[:, NH * b:NH * (b + 1)], hm16, AF.Identity,
                        bias=z16[:, 0:1], scale=q_s[:, b:b + 1])
            r_ps = sps()[:, 0:B_CORE * NH]
            nc.tensor.matmul(r_ps, wk16, qtil[:], start=True, stop=True)
            nc.vector.tensor_copy(
                rp8f[:].rearrange("p (b k) -> p b k", k=137)[:, :, 0:NH],
                r_ps[:].rearrange("p (b h) -> p b h", h=NH))

            # HAM warm-up: dummy matmuls fill the PE while the first X^T
            # chunk is still in flight, so chunk 0 runs at 2.4GHz.
            dmy = psc.tile([H, CW], F32, tag="scps", name="dmy")
            for _ in range(8):
                nc.tensor.matmul(dmy[:], id16, wpack_s[:, 0:CW],
                                 start=True, stop=True)

            # ---------- main chunk pipeline ----------
            E = bigp.tile([H, N2], F16, tag="E")
            sums = smp.tile([H, NCH], F32, tag="sums")
            et = bigp.tile([H, NJ, H], F16, tag="et")
            xn = xnp_p.tile([H, NJ, B_CORE, H], F16, tag="xn")
            u_ps = [psu.tile([H, 4 * H], F32, tag=f"ups{g}", name=f"ups{g}")
                    for g in range(2)]

            for c in range(NCH):
                xTc = xT[:, c, :, :]
                cs = slice(CW * c, CW * (c + 1))
                # scores (chunk 0 in halves so the PE starts DMA-early)
                sc = psc.tile([H, CW], F32, tag="scps", name="sc")
                halves = ((0, HW2), (HW2, CW)) if c == 0 else ((0, CW),)
                for h0, h1 in halves:
                    for b in range(B_CORE):
                        nc.tensor.matmul(sc[:, h0:h1], rp8(b),
                                         xTc[:, b, h0:h1],
                                         start=(b == 0), stop=False)
                    nc.tensor.matmul(
                        sc[:, h0:h1], mnegA9,
                        oh9[:, CW * c + h0:CW * c + h1],
                        start=False, stop=True)
                nc.scalar.activation(
                    E[:, cs], sc[:], AF.Exp,
                    bias=eb[:, 0:1], scale=1.0, accum_out=sums[:, c:c + 1])

                # rebuild X-natural for this chunk (PE transposes, f16 PSUM)
                for i in range(4):
                    j = 4 * c + i
                    xb = tps()
                    for b in range(B_CORE):
                        nc.tensor.transpose(
                            xb[:, H * b:H * (b + 1)],
                            xTc[:, b, H * i:H * (i + 1)], id16)
                    dst = xn[:, j, :, :].rearrange("p a c -> p (a c)")
                    if i % 2 == 0:
                        nc.vector.tensor_copy(dst, xb[:])
                    else:
                        nc.scalar.activation(dst, xb[:], AF.Identity,
                                             bias=z16[:, 0:1], scale=1.0)

                # E^T for this chunk
                ep = tps()[:, 0:4 * H]
                for i in range(4):
                    j = 4 * c + i
                    nc.tensor.transpose(
                        ep[:, H * i:H * (i + 1)], E[:, H * j:H * (j + 1)],
                        id16)
                nc.vector.tensor_copy(
                    et[:, 4 * c:4 * c + 4, :]
                    .rearrange("p a c -> p (a c)"), ep)

                # attnV partial accumulation for this chunk
                for i in range(4):
                    j = 4 * c + i
                    for g in range(2):
                        nc.tensor.matmul(
                            u_ps[g][:], et[:, j, :],
                            xn[:, j, 4 * g:4 * g + 4, :]
                            .rearrange("p a c -> p (a c)"),
                            start=(j == 0), stop=(j == NJ - 1))

            s01 = smp.tile([H, 2], F32, tag="s01")
            nc.vector.tensor_add(s01[:, 0:1], sums[:, 0:1], sums[:, 1:2])
            nc.vector.tensor_add(s01[:, 1:2], sums[:, 2:3], sums[:, 3:4])
            stot = smp.tile([H, 1], F32, tag="stot")
            nc.vector.tensor_add(stot[:], s01[:, 0:1], s01[:, 1:2])
            rcp = smp.tile([H, 1], F32, tag="rcp")
            nc.vector.reciprocal(rcp[:], stot[:])

            # u copies with the 1/sum softmax scale fused (DVE + ScalarE)
            us = smp.tile([H, 2, 4 * H], F16, tag="us")
            nc.vector.tensor_scalar_mul(us[:, 0, :], u_ps[0][:], rcp[:, 0:1])
            nc.scalar.activation(us[:, 1, :], u_ps[1][:], AF.Identity,
                                 bias=0.0, scale=rcp[:, 0:1])

            # ---------- uT, w2 via host-folded per-head weights ----------
            uT = smp.tile([H, B_CORE * NH], F16, tag="uT")
            for b in range(B_CORE):
                o = H * (b % 4)
                ps = tps()[:, 0:H]
                nc.tensor.transpose(ps, us[:, b // 4, o:o + H], id16)
                nc.vector.tensor_copy(
                    uT[:, NH * b:NH * (b + 1)], ps[:, NH * b:NH * (b + 1)])

            # keep the PE's activity monitor busy through the small ops
    
            for _ in range(3):
                nc.tensor.matmul(dmy[:], id16, wpack_s[:, 0:CW],
                                 start=True, stop=True)

            # w2[:,b] = sum_h WLV_h @ uT[:,8b+h] + blc  (WLV host-folded)
            w2_ps = sps()[:, 0:B_CORE]
            for h in range(NH):
                nc.tensor.matmul(
                    w2_ps, wlv16[:, H * h:H * (h + 1)],
                    uT[:].rearrange("p (b h) -> p h b", h=NH)[:, h, :],
                    start=(h == 0), stop=(h == NH - 1))
            w2_s = smp.tile([H, B_CORE], F16, tag="w2_s")
            nc.scalar.activation(
                w2_s[:], w2_ps, AF.Identity, bias=blc_s[:, 0:1], scale=1.0)
            nc.vector.tensor_copy(
                wp8f[:].rearrange("p (b k) -> p b k", k=130)[:, :, 0:1],
                w2_s[:].rearrange("p (b o) -> p b o", o=1))

            # ---------- pointer scores -> tanh -> exp -> norm ----------
            tp = bigp.tile([B_CORE, N2], F16, tag="tp")
            e2 = bigp.tile([B_CORE, N2], F32, tag="e2")
            s2s = smp.tile([B_CORE, NCH], F32, tag="s2s")
            s2p = smp.tile([B_CORE, 2], F32, tag="s2p")
            for c in range(NCH):
                xTc = xT[:, c, :, :]
                cs = slice(CW * c, CW * (c + 1))
                pc = psc.tile([H, CW], F32, tag="scps", name="pc")
                for b in range(B_CORE):
                    nc.tensor.matmul(pc[:], wp8(b), xTc[:, b, :],
                                     start=(b == 0), stop=False)
                nc.tensor.matmul(pc[:], mnegP9, oh9[:, cs],
                                 start=False, stop=True)
                nc.scalar.activation(tp[:, cs], pc[0:B_CORE, :], AF.Tanh)
                nc.scalar.activation(
                    e2[:, cs], tp[:, cs], AF.Exp,
                    bias=0.0, scale=10.0, accum_out=s2s[:, c:c + 1])
                if c == 1:
                    nc.vector.tensor_add(s2p[:, 0:1], s2s[:, 0:1],
                                         s2s[:, 1:2])
                if c == 3:
                    nc.vector.tensor_add(s2p[:, 1:2], s2s[:, 2:3],
                                         s2s[:, 3:4])

            s2tot = smp.tile([B_CORE, 1], F32, tag="s2tot")
            nc.vector.tensor_add(s2tot[:], s2p[:, 0:1], s2p[:, 1:2])
            rcp2 = smp.tile([B_CORE, 1], F32, tag="rcp2")
            nc.vector.reciprocal(rcp2[:], s2tot[:])
            NHALF = 1056
            nc.scalar.activation(e2[:, NHALF:N], e2[:, NHALF:N], AF.Identity,
                                 bias=0.0, scale=rcp2[:, 0:1])
            nc.scalar.dma_start(probs[:, NHALF:N], e2[:, NHALF:N])
            nc.vector.tensor_scalar_mul(e2[:, 0:NHALF], e2[:, 0:NHALF],
                                        rcp2[:, 0:1])
            nc.sync.dma_start(probs[:, 0:NHALF], e2[:, 0:NHALF])

    nc.compile()
    return nc


def _prep_inputs(patch_embeddings, fixed_content_cls, Wq_graph, Wq_first,
                 Wq_last, Wk, Wv, logit_Wk, Wc, bc, last_patch):
    qs = 1.0 / math.sqrt(HD)
    ls = 1.0 / math.sqrt(H)
    f16 = lambda a: np.ascontiguousarray(a, dtype=np.float16)
    f32 = lambda a: np.ascontiguousarray(a, dtype=np.float32)

    hm = np.zeros((H, NH), np.float32)
    for h in range(NH):
        hm[HD * h:HD * (h + 1), h] = qs

    pe = np.asarray(patch_embeddings, dtype=np.float32)
    cls = np.asarray(fixed_content_cls, dtype=np.float32)
    lp = np.asarray(last_patch).astype(np.int64)
    lWk = np.asarray(logit_Wk, dtype=np.float64)
    Wc64 = np.asarray(Wc, dtype=np.float64)

    wpack_base = np.zeros((H, WPACK_W), np.float16)
    wpack_base[:, 0:128] = f16(np.asarray(Wq_graph).T)
    wpack_base[:, 128:256] = f16(np.asarray(Wq_first) + np.asarray(Wq_last)).T
    wpack_base[:, 256:384] = f16(Wk)
    wpack_base[:, 384:512] = np.eye(H, dtype=np.float16)
    wpack_base[:, 512:520] = f16(hm)
    # WLV_h = (ls * lWk^T Wc)[:, hblk] @ Wv[hblk, :]; lhsT slots get WLV_h^T
    Wv64 = np.asarray(Wv, dtype=np.float64)
    M = (lWk.T @ Wc64) * ls
    for h in range(NH):
        hb = slice(HD * h, HD * (h + 1))
        wlv = M[:, hb] @ Wv64[hb, :]
        wpack_base[:, 536 + H * h:536 + H * (h + 1)] = f16(wlv.T)

    blc = f32((lWk.T @ np.asarray(bc, dtype=np.float64) * ls)[:, None])

    in_maps = []
    for c in range(N_CORES):
        bs = slice(B_CORE * c, B_CORE * (c + 1))
        pec = pe[bs]                        # (8, 2000, 128)
        lp_c = lp[bs]
        wpack = wpack_base.copy()
        wpack[:, 520:528] = f16(pec[np.arange(B_CORE), lp_c].T)    # leT
        wpack[:, 528:536] = f16(cls[bs, 0, :].T)                   # clsT

        pad = np.zeros((B_CORE, N2, H), np.float16)
        pad[:, :N, :] = pec
        # (b, chunk, n, c) -> (c, chunk, b, n)
        xtp = np.ascontiguousarray(
            pad.reshape(B_CORE, NCH, CW, H).transpose(3, 1, 0, 2))

        small9 = np.zeros((B_CORE + 1, SM_W), np.float16)
        small9[np.arange(B_CORE), lp_c] = 1.0                      # one-hot
        small9[B_CORE, N:N2] = 1.0                                 # pad cols
        for b in range(B_CORE):
            small9[b, N2 + NH * b:N2 + NH * (b + 1)] = MNEG        # mnegA9
            small9[b, N2 + H + b] = MNEG                           # mnegP9
        small9[B_CORE, N2:N2 + 2 * H] = MNEG                       # pad row

        m = {"xtp": xtp, "wpack": wpack, "small9": small9, "blcd": blc}
        in_maps.append(m)
    return in_maps


def kernel(trace=False, **inputs):
    if "nc" not in _CACHE:
        _CACHE["nc"] = build()
    nc = _CACHE["nc"]
    in_maps = _prep_inputs(**inputs)
    res = run_bass_kernel_spmd(nc, in_maps, list(range(N_CORES)), trace=trace)
    out = np.concatenate(
        [res.results[c]["probs"].astype(np.float32) for c in range(N_CORES)],
        axis=0)
    if trace:
        return out, res
    return out


# revision 22
# speedup vs baseline: 1.0521x; 1.0400x over previous
"""Trainium2 Bass kernel for nn_Decoder_74835510165950 (sparse_attention).

Single-query attention decoder over B=64, N=2000, H=128, 8 heads.
Data-parallel over 8 NeuronCores: 8 batches per core.

v4 design:
  - Only X^T fp16 is DMA'd (4.2MB/core, N padded to 2048, chunk-major
    -> 8KB contiguous runs, ~350GB/s). Small tensors go FIRST on the
    same queue so the Q-path isn't starved behind the bulk load.
  - X-natural (for attnV) is rebuilt on device: 8 PE transposes into one
    fp16 [128,1024] PSUM bank per 128-col n-tile, then a single copy,
    alternating Scalar/Vector engines so neither serializes the PE.
  - All 8 batches packed per PSUM tile (scores rows 8b..8b+8, pointer
    row b); -60000 masking via one extra matmul per chunk with a host
    one-hot rhs; a 9th row masks the 48 pad columns.
  - fp16 streams everywhere (PSUM fp32); softmax exp biased by -8 so E
    fits fp16; 1/sum applied to u after attnV; logit_Wk^T@Wc and
    logit_Wk^T@bc folded on the host so the output head is one matmul.
  - Fully chunk-pipelined: DMA -> scores -> exp -> xnat rebuild + E^T ->
    attnV per 512-col chunk; only the small chain + pointer phase trail
    the last chunk.
"""
import sys

if "/opt/trn_rl_repo" not in sys.path:
    sys.path.insert(0, "/opt/trn_rl_repo")

import math
import numpy as np

import concourse.bass as bass
import concourse.tile as tile
from concourse import bacc, mybir
from concourse.bass_utils import run_bass_kernel_spmd

F32 = mybir.dt.float32
F16 = mybir.dt.float16

N_CORES = 8
B_CORE = 8          # batches per core
N = 2000
N2 = 2048           # padded
H = 128
NH = 8              # heads
HD = 16             # head dim
NCH = 4             # chunks
CW = 512            # chunk width (= one psum bank of fp32)
NJ = 16             # 128-col n-tiles (N2 / 128)
EXP_BIAS = -8.0     # uniform shift inside softmax exp (cancels in ratio)
MNEG = -60000.0

# wpack column layout (all fp16, [128, x])
_WCOLS = {"wqgT": 0, "wsumT": 128, "wk": 256, "id": 384, "hm": 512,
          "leT": 520, "clsT": 528, "wlv": 536}
WPACK_W = 536 + 8 * 128
SM_W = N2 + 2 * H   # small9 row width

_CACHE = {}


def build():
    nc = bacc.Bacc("TRN2", target_bir_lowering=False, debug=False)

    xtp = nc.dram_tensor("xtp", [H, NCH, B_CORE, CW], F16, kind="ExternalInput")
    wpack = nc.dram_tensor("wpack", [H, WPACK_W], F16, kind="ExternalInput")
    small9 = nc.dram_tensor("small9", [B_CORE + 1, SM_W], F16,
                            kind="ExternalInput")
    blcd = nc.dram_tensor("blcd", [H, 1], F32, kind="ExternalInput")

    probs = nc.dram_tensor("probs", [B_CORE, N], F32, kind="ExternalOutput")

    AF = mybir.ActivationFunctionType

    with tile.TileContext(nc) as tc:
        with (
            tc.tile_pool(name="wts", bufs=1) as wts,
            tc.tile_pool(name="xt", bufs=1) as xtp_p,
            tc.tile_pool(name="xn", bufs=1) as xnp_p,
            tc.tile_pool(name="big", bufs=1) as bigp,
            tc.tile_pool(name="sm", bufs=1) as smp,
            tc.tile_pool(name="ps_sc", bufs=2, space="PSUM") as psc,
            tc.tile_pool(name="ps_tr", bufs=3, space="PSUM") as pst,
            tc.tile_pool(name="ps_u", bufs=1, space="PSUM") as psu,
            tc.tile_pool(name="ps_sm", bufs=1, space="PSUM") as pss,
        ):
            # ---------- DMAs: smalls first, then X^T chunks ----------
            wpack_s = wts.tile([H, WPACK_W], F16, tag="wpack")
            nc.sync.dma_start(wpack_s[:], wpack[:])
            small9_s = wts.tile([B_CORE + 1, SM_W], F16, tag="small9")
            nc.sync.dma_start(small9_s[:], small9[:])
            blc_s = wts.tile([H, 1], F32, tag="blc")
            nc.sync.dma_start(blc_s[:], blcd[:])

            def wcol(name, w):
                c0 = _WCOLS[name]
                return wpack_s[:, c0:c0 + w]

            wqgT16, wsumT16 = wcol("wqgT", H), wcol("wsumT", H)
            wk16 = wcol("wk", H)
            wlv16 = wcol("wlv", NH * H)
            id16 = wcol("id", H)
            hm16 = wcol("hm", NH)
            leT16 = wcol("leT", B_CORE)
            clsT16 = wcol("clsT", B_CORE)
            oh9 = small9_s[:, 0:N2]
            mnegA9 = small9_s[:, N2:N2 + H]
            mnegP9 = small9_s[:, N2 + H:N2 + 2 * H]

            xT = xtp_p.tile([H, NCH, B_CORE, CW], F16, tag="xT")
            HW2 = CW // 2
            nc.sync.dma_start(xT[:, 0, :, 0:HW2], xtp[:, 0, :, 0:HW2])
            nc.sync.dma_start(xT[:, 0, :, HW2:CW], xtp[:, 0, :, HW2:CW])
            for c in range(1, NCH):
                nc.sync.dma_start(xT[:, c, :, :], xtp[:, c, :, :])

            def sps():
                return pss.tile([H, H], F32, tag="smps", name="smps")

            def tps():
                return pst.tile([H, 8 * H], F16, tag="tps", name="tps")

            # ---------- Q path (runs during X DMA) ----------
            # flat stationaries: window b = cols 129b..129b+128. rp8f holds
            # R cols at 137b+h (=> window-local 8b+h); wp8f holds w2 col at
            # 130b (=> window-local b). One strided copy builds each.
            rp8f = smp.tile([H, 137 * B_CORE], F16, tag="rp8f")
            nc.gpsimd.memset(rp8f[:], 0.0)
            wp8f = smp.tile([H, 130 * B_CORE], F16, tag="wp8f")
            nc.gpsimd.memset(wp8f[:], 0.0)

            def rp8(b):
                return rp8f[:, 129 * b:129 * b + H]

            def wp8(b):
                return wp8f[:, 129 * b:129 * b + H]
            eb = smp.tile([H, 1], F32, tag="eb")
            nc.vector.memset(eb[:], EXP_BIAS)
            z16 = smp.tile([H, 1], F16, tag="z16")
            nc.vector.memset(z16[:], 0.0)

            q_ps = sps()[:, 0:B_CORE]
            nc.tensor.matmul(q_ps, wqgT16, clsT16, start=True, stop=False)
            nc.tensor.matmul(q_ps, wsumT16, leT16, start=False, stop=True)
            q_s = smp.tile([H, B_CORE], F32, tag="q_s")
            nc.vector.tensor_copy(q_s[:], q_ps)

            qtil = smp.tile([H, B_CORE * NH], F16, tag="qtil")
            for b in range(B_CORE):
                if b % 2 == 0:
                    nc.vector.tensor_scalar_mul(
                        qtil[:, NH * b:NH * (b + 1)], hm16, q_s[:, b:b + 1])
                else:
                    nc.scalar.activation(
                        qtil[:, NH * b:NH * (b + 1)], hm16, AF.Identity,
                        bias=z16[:, 0:1], scale=q_s[:, b:b + 1])
            r_ps = sps()[:, 0:B_CORE * NH]
            nc.tensor.matmul(r_ps, wk16, qtil[:], start=True, stop=True)
            nc.vector.tensor_copy(
                rp8f[:].rearrange("p (b k) -> p b k", k=137)[:, :, 0:NH],
                r_ps[:].rearrange("p (b h) -> p b h", h=NH))

            # HAM warm-up: dummy matmuls fill the PE while the first X^T
            # chunk is still in flight, so chunk 0 runs at 2.4GHz.
            dmy = psc.tile([H, CW], F32, tag="scps", name="dmy")
            for _ in range(8):
                nc.tensor.matmul(dmy[:], id16, wpack_s[:, 0:CW],
                                 start=True, stop=True)

            # ---------- main chunk pipeline ----------
            E = bigp.tile([H, N2], F16, tag="E")
            sums = smp.tile([H, NCH], F32, tag="sums")
            et = bigp.tile([H, NJ, H], F16, tag="et")
            xn = xnp_p.tile([H, NJ, B_CORE, H], F16, tag="xn")
            u_ps = [psu.tile([H, 4 * H], F32, tag=f"ups{g}", name=f"ups{g}")
                    for g in range(2)]

            for c in range(NCH):
                xTc = xT[:, c, :, :]
                cs = slice(CW * c, CW * (c + 1))
                # scores (chunk 0 in halves so the PE starts DMA-early)
                sc = psc.tile([H, CW], F32, tag="scps", name="sc")
                halves = ((0, HW2), (HW2, CW)) if c == 0 else ((0, CW),)
                for h0, h1 in halves:
                    for b in range(B_CORE):
                        nc.tensor.matmul(sc[:, h0:h1], rp8(b),
                                         xTc[:, b, h0:h1],
                                         start=(b == 0), stop=False)
                    nc.tensor.matmul(
                        sc[:, h0:h1], mnegA9,
                        oh9[:, CW * c + h0:CW * c + h1],
                        start=False, stop=True)
                nc.scalar.activation(
                    E[:, cs], sc[:], AF.Exp,
                    bias=eb[:, 0:1], scale=1.0, accum_out=sums[:, c:c + 1])

                # rebuild X-natural for this chunk (PE transposes, f16 PSUM)
                for i in range(4):
                    j = 4 * c + i
                    xb = tps()
                    for b in range(B_CORE):
                        nc.tensor.transpose(
                            xb[:, H * b:H * (b + 1)],
                            xTc[:, b, H * i:H * (i + 1)], id16)
                    dst = xn[:, j, :, :].rearrange("p a c -> p (a c)")
                    if i % 2 == 0:
                        nc.vector.tensor_copy(dst, xb[:])
                    else:
                        nc.scalar.activation(dst, xb[:], AF.Identity,
                                             bias=z16[:, 0:1], scale=1.0)

                # E^T for this chunk
                ep = tps()[:, 0:4 * H]
                for i in range(4):
                    j = 4 * c + i
                    nc.tensor.transpose(
                        ep[:, H * i:H * (i + 1)], E[:, H * j:H * (j + 1)],
                        id16)
                nc.vector.tensor_copy(
                    et[:, 4 * c:4 * c + 4, :]
                    .rearrange("p a c -> p (a c)"), ep)

                # attnV partial accumulation for this chunk (g-outer so
                # same-bank matmuls stay back-to-back and pipeline)
                for g in range(2):
                    for i in range(4):
                        j = 4 * c + i
                        nc.tensor.matmul(
                            u_ps[g][:], et[:, j, :],
                            xn[:, j, 4 * g:4 * g + 4, :]
                            .rearrange("p a c -> p (a c)"),
                            start=(j == 0), stop=(j == NJ - 1))

            # unscaled u copies fire the moment attnV stops (no rcp dep)
            us = smp.tile([H, 2, 4 * H], F16, tag="us")
            nc.vector.tensor_copy(us[:, 0, :], u_ps[0][:])
            nc.scalar.activation(us[:, 1, :], u_ps[1][:], AF.Identity,
                                 bias=0.0, scale=1.0)

            s01 = smp.tile([H, 2], F32, tag="s01")
            nc.vector.tensor_add(s01[:, 0:1], sums[:, 0:1], sums[:, 1:2])
            nc.vector.tensor_add(s01[:, 1:2], sums[:, 2:3], sums[:, 3:4])
            stot = smp.tile([H, 1], F32, tag="stot")
            nc.vector.tensor_add(stot[:], s01[:, 0:1], s01[:, 1:2])
            rcp = smp.tile([H, 1], F32, tag="rcp")
            nc.vector.reciprocal(rcp[:], stot[:])
            diag16 = smp.tile([H, H], F16, tag="diag16")
            nc.vector.tensor_scalar_mul(diag16[:], id16, rcp[:, 0:1])

            # uT[:,8b:8b+8] = us_q^T @ diag(rcp)[:,8b:8b+8] -- regular
            # matmuls (the transpose datapath ignores rhs values), all into
            # one f32 PSUM tile, one copy out, softmax scale fused for free.
            uT = smp.tile([H, B_CORE * NH], F16, tag="uT")
            ut_ps = sps()[:, 0:B_CORE * NH]
            for b in range(B_CORE):
                o = H * (b % 4)
                nc.tensor.matmul(
                    ut_ps[:, NH * b:NH * (b + 1)], us[:, b // 4, o:o + H],
                    diag16[:, NH * b:NH * (b + 1)], start=True, stop=True)
            nc.vector.tensor_copy(uT[:], ut_ps)

            # w2[:,b] = sum_h WLV_h @ uT[:,8b+h] + blc  (WLV host-folded)
            w2_ps = sps()[:, 0:B_CORE]
            for h in range(NH):
                nc.tensor.matmul(
                    w2_ps, wlv16[:, H * h:H * (h + 1)],
                    uT[:].rearrange("p (b h) -> p h b", h=NH)[:, h, :],
                    start=(h == 0), stop=(h == NH - 1))
            w2_s = smp.tile([H, B_CORE], F16, tag="w2_s")
            nc.scalar.activation(
                w2_s[:], w2_ps, AF.Identity, bias=blc_s[:, 0:1], scale=1.0)
            nc.vector.tensor_copy(
                wp8f[:].rearrange("p (b k) -> p b k", k=130)[:, :, 0:1],
                w2_s[:].rearrange("p (b o) -> p b o", o=1))

            # ---------- pointer scores -> tanh -> exp -> norm ----------
            tp = bigp.tile([B_CORE, N2], F16, tag="tp")
            e2 = bigp.tile([B_CORE, N2], F32, tag="e2")
            s2s = smp.tile([B_CORE, NCH + 1], F32, tag="s2s")
            s2p = smp.tile([B_CORE, 2], F32, tag="s2p")
            for c in range(NCH):
                xTc = xT[:, c, :, :]
                cs = slice(CW * c, CW * (c + 1))
                pc = psc.tile([H, CW], F32, tag="scps", name="pc")
                halves = ((0, HW2), (HW2, CW)) if c == NCH - 1 else ((0, CW),)
                for hi, (h0, h1) in enumerate(halves):
                    for b in range(B_CORE):
                        nc.tensor.matmul(pc[:, h0:h1], wp8(b),
                                         xTc[:, b, h0:h1],
                                         start=(b == 0), stop=False)
                    nc.tensor.matmul(
                        pc[:, h0:h1], mnegP9,
                        oh9[:, CW * c + h0:CW * c + h1],
                        start=False, stop=True)
                    hs = slice(CW * c + h0, CW * c + h1)
                    nc.scalar.activation(tp[:, hs], pc[0:B_CORE, h0:h1],
                                         AF.Tanh)
                    nc.scalar.activation(
                        e2[:, hs], tp[:, hs], AF.Exp, bias=0.0, scale=10.0,
                        accum_out=s2s[:, c + hi:c + hi + 1])
                if c == 1:
                    nc.vector.tensor_add(s2p[:, 0:1], s2s[:, 0:1],
                                         s2s[:, 1:2])
                if c == 3:
                    nc.vector.tensor_add(s2p[:, 1:2], s2s[:, 3:4],
                                         s2s[:, 4:5])
                if c == 2:
                    nc.vector.tensor_add(s2p[:, 0:1], s2p[:, 0:1],
                                         s2s[:, 2:3])

            s2tot = smp.tile([B_CORE, 1], F32, tag="s2tot")
            nc.vector.tensor_add(s2tot[:], s2p[:, 0:1], s2p[:, 1:2])
            rcp2 = smp.tile([B_CORE, 1], F32, tag="rcp2")
            nc.vector.reciprocal(rcp2[:], s2tot[:])
            NHALF = 1056
            nc.scalar.activation(e2[:, NHALF:N], e2[:, NHALF:N], AF.Identity,
                                 bias=0.0, scale=rcp2[:, 0:1])
            nc.scalar.dma_start(probs[:, NHALF:N], e2[:, NHALF:N])
            nc.vector.tensor_scalar_mul(e2[:, 0:NHALF], e2[:, 0:NHALF],
                                        rcp2[:, 0:1])
            nc.sync.dma_start(probs[:, 0:NHALF], e2[:, 0:NHALF])

    nc.compile()
    return nc


def _prep_inputs(patch_embeddings, fixed_content_cls, Wq_graph, Wq_first,
                 Wq_last, Wk, Wv, logit_Wk, Wc, bc, last_patch):
    qs = 1.0 / math.sqrt(HD)
    ls = 1.0 / math.sqrt(H)
    f16 = lambda a: np.ascontiguousarray(a, dtype=np.float16)
    f32 = lambda a: np.ascontiguousarray(a, dtype=np.float32)

    hm = np.zeros((H, NH), np.float32)
    for h in range(NH):
        hm[HD * h:HD * (h + 1), h] = qs

    pe = np.asarray(patch_embeddings, dtype=np.float32)
    cls = np.asarray(fixed_content_cls, dtype=np.float32)
    lp = np.asarray(last_patch).astype(np.int64)
    lWk = np.asarray(logit_Wk, dtype=np.float64)
    Wc64 = np.asarray(Wc, dtype=np.float64)

    wpack_base = np.zeros((H, WPACK_W), np.float16)
    wpack_base[:, 0:128] = f16(np.asarray(Wq_graph).T)
    wpack_base[:, 128:256] = f16(np.asarray(Wq_first) + np.asarray(Wq_last)).T
    wpack_base[:, 256:384] = f16(Wk)
    wpack_base[:, 384:512] = np.eye(H, dtype=np.float16)
    wpack_base[:, 512:520] = f16(hm)
    # WLV_h = (ls * lWk^T Wc)[:, hblk] @ Wv[hblk, :]; lhsT slots get WLV_h^T
    Wv64 = np.asarray(Wv, dtype=np.float64)
    M = (lWk.T @ Wc64) * ls
    for h in range(NH):
        hb = slice(HD * h, HD * (h + 1))
        wlv = M[:, hb] @ Wv64[hb, :]
        wpack_base[:, 536 + H * h:536 + H * (h + 1)] = f16(wlv.T)

    blc = f32((lWk.T @ np.asarray(bc, dtype=np.float64) * ls)[:, None])

    in_maps = []
    for c in range(N_CORES):
        bs = slice(B_CORE * c, B_CORE * (c + 1))
        pec = pe[bs]                        # (8, 2000, 128)
        lp_c = lp[bs]
        wpack = wpack_base.copy()
        wpack[:, 520:528] = f16(pec[np.arange(B_CORE), lp_c].T)    # leT
        wpack[:, 528:536] = f16(cls[bs, 0, :].T)                   # clsT

        pad = np.zeros((B_CORE, N2, H), np.float16)
        pad[:, :N, :] = pec
        # (b, chunk, n, c) -> (c, chunk, b, n)
        xtp = np.ascontiguousarray(
            pad.reshape(B_CORE, NCH, CW, H).transpose(3, 1, 0, 2))

        small9 = np.zeros((B_CORE + 1, SM_W), np.float16)
        small9[np.arange(B_CORE), lp_c] = 1.0                      # one-hot
        small9[B_CORE, N:N2] = 1.0                                 # pad cols
        for b in range(B_CORE):
            small9[b, N2 + NH * b:N2 + NH * (b + 1)] = MNEG        # mnegA9
            small9[b, N2 + H + b] = MNEG                           # mnegP9
        small9[B_CORE, N2:N2 + 2 * H] = MNEG                       # pad row

        m = {"xtp": xtp, "wpack": wpack, "small9": small9, "blcd": blc}
        in_maps.append(m)
    return in_maps


def kernel(trace=False, **inputs):
    if "nc" not in _CACHE:
        _CACHE["nc"] = build()
    nc = _CACHE["nc"]
    in_maps = _prep_inputs(**inputs)
    res = run_bass_kernel_spmd(nc, in_maps, list(range(N_CORES)), trace=trace)
    out = np.concatenate(
        [res.results[c]["probs"].astype(np.float32) for c in range(N_CORES)],
        axis=0)
    if trace:
        return out, res
    return out
